# revision 19
# baseline (speedup 1.0000x reference)
"""BandSplit (gather -> per-band MLP -> scatter-add OLA -> /ola) on 8 TRN2 cores.

Strategy
--------
The whole reference computation is linear in x, so on the host we fold the
per-band pre/post weights, melbank weights, mask, scatter-add and /ola into a
single banded matrix A of shape (C*F, C*F) mapping the (c, f) spectrum of one
(b, t) token to the output spectrum (see _fold_matrix).  The device kernel is
a banded matmul, data-parallel over the 4096 (b, t) tokens across the 8
NeuronCores (512 tokens/core, 4 chunks of 128) with zero cross-core traffic.

v2 layout (vs the v0 baseline):
 * Host pre-transposes x into contraction-major layout (partition = 64
   consecutive f values x 2 input channels), so the PE transposes and the
   gpsimd cast-DMAs are gone entirely and both input channels contract in a
   single matmul pass (halves the number of PE passes; 64-row windows are
   narrower than two 128-row passes: 4764 vs 6956 cols per token chunk).
 * x for low-frequency groups 0..13 ships as fp8 e3m4 (x2 pre-scale folded
   into A) to cut HBM read traffic; high groups stay bf16 where the wide
   bands accumulate too many terms for fp8 (measured rel-err 0.011 vs the
   2e-2 budget).  A stays bf16.
 * All loads are fat HWDGE DMAs on the sync ring; stores go on the scalar
   ring so they never head-of-line block loads.
 * PSUM holds one token-chunk of output (2050 interleaved f32 cols, 5 banks);
   drains are bank-granular, alternating DVE/ACT, so the next chunk's
   matmuls only wait for the one bank they touch.
"""

import numpy as np

_P = 128
_G = 64            # f rows per partition group (x2 channels = 128 partitions)
_C = 2
_F = 1025
_NG = 17           # groups cover f = 0..1087 (1025 real + bias row 1025)
_FP8_GROUPS = 14   # groups 0..13 in e3m4, 14..16 in bf16
_FP8_SCALE = 2.0
_TOK_CORE = 512    # tokens per core
_TCH = 4           # token chunks of 128
_N_CORES = 8


def _fold_matrix(pre_w, pre_b, post_w, post_b, idx, melw, mask, ola_window):
    """Fold the full reference computation into (A, const).

    A: (C, F, C, F) with out[co, fo] = sum_{ci, fi} x[ci, fi] * A[ci, fi, co, fo]
    const: (C, F) additive constant from the biases.
    """
    K, W = idx.shape
    C = _C
    F = ola_window.shape[0]

    pre_w = np.asarray(pre_w, np.float64)
    post_w = np.asarray(post_w, np.float64)
    pre_b = np.asarray(pre_b, np.float64)
    post_b = np.asarray(post_b, np.float64)
    wts = (np.asarray(melw, np.float64) * np.asarray(mask, np.float64))
    msk = np.asarray(mask, np.float64)
    idx = np.asarray(idx)

    M = np.einsum('kio,koj->kij', pre_w, post_w).reshape(K, W, C, W, C)
    vals = M * wts[:, :, None, None, None] * msk[:, None, None, :, None]

    fin = idx[:, :, None, None, None].astype(np.int64)
    fout = idx[:, None, None, :, None].astype(np.int64)
    cin = np.arange(C)[None, None, :, None, None]
    cout = np.arange(C)[None, None, None, None, :]
    flat = ((cin * F + fin) * C + cout) * F + fout
    A = np.bincount(
        np.broadcast_to(flat, vals.shape).ravel(), weights=vals.ravel(),
        minlength=C * F * C * F,
    ).reshape(C, F, C, F)
    A /= ola_window[None, None, None, :]

    bv = (np.einsum('ko,koj->kj', pre_b, post_w) + post_b).reshape(K, W, C)
    bv = bv * msk[:, :, None]
    cflat = (np.arange(C)[None, None, :] * F + idx[:, :, None]).astype(np.int64)
    const = np.bincount(
        np.broadcast_to(cflat, bv.shape).ravel(), weights=bv.ravel(),
        minlength=C * F,
    ).reshape(C, F)
    const /= ola_window[None, :]
    return A, const


def _plan(A, const):
    """Build the banded layout: per-group windows, packed A, segment lists.

    Rows of group j (128 partitions): p = 2*(f - 64j) + ci for f in
    [64j, 64j+64), both channels.  f == 1025 is the bias row (x column == 1).
    Output columns are channel-interleaved: col = 2*fo + co, 2050 total.
    """
    F, C, NG, G = _F, _C, _NG, _G
    # Ap[ci, f, co, fo] over padded f rows (F+1 rows: bias at F)
    Ap = np.zeros((C, NG * G, C, F), np.float64)
    Ap[:, :F] = A
    Ap[0, F] = const

    nzrow = (Ap != 0).any(axis=(0, 2))          # (NG*G, F) over (f, fo)
    wins = []
    for j in range(NG):
        cols = nzrow[j * G:(j + 1) * G].any(axis=0)
        nzc = np.nonzero(cols)[0]
        lo, hi = (int(nzc[0]), int(nzc[-1]) + 1) if len(nzc) else (0, 1)
        wins.append((lo, hi))
    cov = np.zeros(F, bool)
    for lo, hi in wins:
        cov[lo:hi] = True
    assert cov.all(), 'window coverage hole'

    # packed A: [128, TW] with 16-col-aligned per-group blocks
    offs, tw = [], 0
    for j in range(NG):
        offs.append(tw)
        tw += (2 * (wins[j][1] - wins[j][0]) + 15) // 16 * 16
    import ml_dtypes
    ab = np.zeros((_P, tw), ml_dtypes.bfloat16)
    for j in range(NG):
        lo, hi = wins[j]
        blk = Ap[:, j * G:(j + 1) * G, :, lo:hi]       # (ci, 64, co, w)
        blk = blk.transpose(1, 0, 3, 2).reshape(_P, -1)  # p=(f,ci), col=(fo,co)
        if j < _FP8_GROUPS:
            blk = blk / _FP8_SCALE                     # undo x pre-scale
        ab[:, offs[j]:offs[j] + 2 * (hi - lo)] = blk

    # matmul segments per group, split at 512-col PSUM bank boundaries
    segs = []                                          # [(j, s, e)] in order
    for j in range(NG):
        lo2, hi2 = 2 * wins[j][0], 2 * wins[j][1]
        s = lo2
        while s < hi2:
            e = min(hi2, (s // 512 + 1) * 512)
            segs.append((j, s, e))
            s = e
    # first/last toucher of each bank (for start/stop flags), per chunk
    bank_first, bank_last = {}, {}
    for i, (j, s, e) in enumerate(segs):
        b = s // 512
        bank_first.setdefault(b, i)
        bank_last[b] = i
    return wins, offs, tw, ab, segs, bank_first, bank_last


_PROGRAM_CACHE = {}


def _build_program(tw, wins, offs, segs, bank_first, bank_last):
    import concourse.bass as bass  # noqa: F401
    import concourse.tile as tile
    import concourse.mybir as mybir
    from concourse import bacc
    from concourse.masks import make_identity

    f32 = mybir.dt.float32
    bf16 = mybir.dt.bfloat16
    f16 = mybir.dt.float16
    fp8 = mybir.dt.float8e3
    P = _P
    NG, TCH = _NG, _TCH
    N8 = _FP8_GROUPS
    N16 = NG - N8
    W_OUT = 2 * _F                      # 2050 interleaved output cols

    nc = bacc.Bacc("TRN2", target_bir_lowering=False, debug=False,
                   num_devices=_N_CORES)
    xs8 = nc.dram_tensor("xs8", [P, N8 * _TOK_CORE], fp8, kind="ExternalInput")
    xs16 = nc.dram_tensor("xs16", [P, N16 * _TOK_CORE], bf16,
                          kind="ExternalInput")
    ab = nc.dram_tensor("ab", [P, tw], bf16, kind="ExternalInput")
    y = nc.dram_tensor("y", [TCH, P, W_OUT], f16, kind="ExternalOutput")

    # loads stream on three DMA rings concurrently, byte-balanced (sync:
    # x fp8; scalar: A groups 0-11; gpsimd: A groups 12-16 then x bf16);
    # stores alternate between the sync and gpsimd rings
    X8_SPLITS = [(0, 2), (2, 5), (5, 8), (8, 11), (11, 14)]
    A_SPLITS_SC = [(0, 3), (3, 6), (6, 9), (9, 12)]
    A_SPLITS_GP = [(12, 15), (15, 17)]
    NBANK = (W_OUT + 511) // 512            # 5 PSUM banks (last holds 2 cols)

    with tile.TileContext(nc) as tc:
        with (
            tc.tile_pool(name="xpool", bufs=1) as xpool,
            tc.tile_pool(name="apool", bufs=1) as apool,
            tc.tile_pool(name="opool", bufs=2) as opool,
            tc.tile_pool(name="idpool", bufs=1) as idpool,
            tc.tile_pool(name="psa", bufs=1, space="PSUM") as psa,
            tc.tile_pool(name="psb", bufs=2, space="PSUM") as psb,
        ):
            x8t = xpool.tile([P, N8 * _TOK_CORE], fp8, name="x8")
            x16t = xpool.tile([P, N16 * _TOK_CORE], bf16, name="x16")
            abt = apool.tile([P, tw], bf16, name="abt")
            ident = idpool.tile([P, P], bf16, name="ident")
            make_identity(nc, ident[:])

            S = _TOK_CORE
            for g0, g1 in X8_SPLITS:
                nc.sync.dma_start(x8t[:, g0 * S:g1 * S], xs8[:, g0 * S:g1 * S])
            for a0, a1 in A_SPLITS_SC:
                nc.scalar.dma_start(abt[:, offs[a0]:offs[a1]],
                                    ab[:, offs[a0]:offs[a1]])
            for a0, a1 in A_SPLITS_GP:
                o1 = tw if a1 >= NG else offs[a1]
                nc.gpsimd.dma_start(abt[:, offs[a0]:o1], ab[:, offs[a0]:o1])
            nc.gpsimd.dma_start(x16t[:], xs16[:])

            # PE warmup: >=3.4us of continuous matmuls trips the HAM clock
            # gate to 2.4 GHz while the DMAs land
            warm = psa.tile([P, P], f32, tag="warm", name="warm")
            for _ in range(36):
                nc.tensor.matmul(warm[:], ident[:], ident[:],
                                 start=True, stop=True)

            def lhsT(j, t):
                if j < N8:
                    return x8t[:, j * S + t * P:j * S + (t + 1) * P]
                return x16t[:, (j - N8) * S + t * P:(j - N8) * S + (t + 1) * P]

            # one PSUM tile per 512-col bank: drain deps are bank-granular.
            # Banks 3/4 finish at the very end of each chunk, so they get
            # double buffers; banks 0-2 drain mid-chunk and single-buffer.
            def bank_tile(t, b):
                w = min(512, W_OUT - b * 512)
                pool = psb if b >= 3 else psa
                return pool.tile([P, w], f32, tag=f"ptb{b}",
                                 name=f"pt_{t}_{b}")

            segs_by_group = {}
            for i, (j, s, e) in enumerate(segs):
                segs_by_group.setdefault(j, []).append((i, s, e))
            bank_total = {}
            for (j, s, e) in segs:
                bank_total[s // 512] = bank_total.get(s // 512, 0) + 1

            # interleaved chunk schedule: while chunk 0's tail groups wait on
            # the input DMAs, run chunk 1/2's early groups whose PSUM banks
            # chunk 0 has already retired (bank b of chunk t may start only
            # after chunk t-1's bank b drained: b0 retires at g4, b1 at g8,
            # b2 at g13, b3/b4 are double-buffered).
            SCHED = [(0, 0, 12), (1, 0, 7), (2, 0, 3), (0, 12, NG),
                     (1, 7, NG), (2, 3, NG), (3, 0, NG)]
            assert sorted((t, j) for t, j0, j1 in SCHED
                          for j in range(j0, j1)) == \
                sorted((t, j) for t in range(TCH) for j in range(NG))
            # static safety: chunk t's first touch of bank b must come after
            # the bank's previous user fully retired (PE FIFO deadlocks
            # otherwise, since the drain it waits on would be emitted later)
            emit_order = [(t, s // 512) for (t, j0, j1) in SCHED
                          for j in range(j0, j1)
                          for (i, s, e) in segs_by_group[j]]
            first_touch, retire_pos, cnt = {}, {}, {}
            for pos, (t, b) in enumerate(emit_order):
                first_touch.setdefault((t, b), pos)
                cnt[(t, b)] = cnt.get((t, b), 0) + 1
                if cnt[(t, b)] == bank_total[b]:
                    retire_pos[(t, b)] = pos
            nbufs = {0: 1, 1: 1, 2: 1, 3: 2, 4: 2}
            for (t, b), pos in first_touch.items():
                if t >= nbufs[b]:
                    assert retire_pos[(t - nbufs[b], b)] < pos, (t, b)

            pts, ots = {}, {}
            bank_done = {}
            drained = {t: set() for t in range(TCH)}
            drain_eng = {0: 'v', 1: 's', 2: 'v', 3: 's', 4: 'v'}

            for (t, j0, j1) in SCHED:
                for j in range(j0, j1):
                    lo2 = 2 * wins[j][0]
                    o = offs[j]
                    for (i, s, e) in segs_by_group[j]:
                        b = s // 512
                        if (t, b) not in pts:
                            pts[(t, b)] = bank_tile(t, b)
                        nc.tensor.matmul(
                            pts[(t, b)][:, s - b * 512:e - b * 512],
                            lhsT(j, t),
                            abt[:, o + s - lo2:o + e - lo2],
                            start=(bank_first[b] == i),
                            stop=(bank_last[b] == i),
                        )
                        bank_done[(t, b)] = bank_done.get((t, b), 0) + 1
                        if bank_done[(t, b)] < bank_total[b]:
                            continue
                        # bank (t, b) retired: drain it now, store halves as
                        # soon as their banks are all in SBUF
                        if t not in ots:
                            ots[t] = opool.tile([P, W_OUT], f16, tag="out",
                                                name=f"out_{t}")
                        dst = ots[t][:, b * 512:b * 512 + min(
                            512, W_OUT - b * 512)]
                        if drain_eng[b] == 'v':
                            nc.vector.tensor_copy(dst, pts[(t, b)][:])
                        else:
                            nc.scalar.copy(dst, pts[(t, b)][:])
                        drained[t].add(b)
                        if b in (0, 1) and {0, 1} <= drained[t]:
                            nc.sync.dma_start(y[t, :, 0:1024],
                                              ots[t][:, 0:1024])
                        if {2, 3, 4} <= drained[t]:
                            nc.gpsimd.dma_start(y[t, :, 1024:W_OUT],
                                                ots[t][:, 1024:W_OUT])

    nc.compile()
    return nc


def kernel(**inputs):
    import ml_dtypes

    x = np.ascontiguousarray(np.asarray(inputs["x"], np.float32))
    B, C, T, F = x.shape
    assert (B, C, F) == (4, _C, _F), (B, C, F)
    TS = T // _N_CORES

    A, const = _fold_matrix(
        inputs["pre_w"], inputs["pre_b"], inputs["post_w"], inputs["post_b"],
        inputs["idx"], inputs["melw"], inputs["mask"], inputs["ola_window"],
    )
    wins, offs, tw, ab, segs, bank_first, bank_last = _plan(A, const)

    key = (tw, tuple(wins))
    if key not in _PROGRAM_CACHE:
        _PROGRAM_CACHE[key] = _build_program(tw, wins, offs, segs,
                                             bank_first, bank_last)
    nc = _PROGRAM_CACHE[key]

    # host pre-shard: contraction-major x layout per core.
    # xq[ci, f, b, t] with f padded to 1088 (bias row at f=1025 == 1.0)
    NGG = _NG * _G
    xq = np.zeros((_C, NGG, B, T), np.float32)
    xq[:, :F] = x.transpose(1, 3, 0, 2)
    xq[0, F] = 1.0
    # [NG, G, C, B, T] -> partitions p = 2*f_off + ci
    xq = xq.reshape(_C, _NG, _G, B, T).transpose(1, 2, 0, 3, 4)
    x8 = (xq[:_FP8_GROUPS] * _FP8_SCALE).astype(ml_dtypes.float8_e3m4)
    x16 = xq[_FP8_GROUPS:].astype(ml_dtypes.bfloat16)

    in_maps = []
    for m in range(_N_CORES):
        sl8 = x8[:, :, :, :, m * TS:(m + 1) * TS]      # (N8, G, C, B, TS)
        sl16 = x16[:, :, :, :, m * TS:(m + 1) * TS]
        in_maps.append({
            "xs8": np.ascontiguousarray(
                sl8.reshape(_FP8_GROUPS, _P // 2 // 1, _C, _TOK_CORE)
                   .reshape(_FP8_GROUPS, _G * _C, _TOK_CORE)
                   .transpose(1, 0, 2).reshape(_P, -1)),
            "xs16": np.ascontiguousarray(
                sl16.reshape(_NG - _FP8_GROUPS, _G * _C, _TOK_CORE)
                    .transpose(1, 0, 2).reshape(_P, -1)),
            "ab": ab,
        })

    try:
        import antenv.axon_hooks  # noqa: F401
    except ImportError:
        import sys
        import types
        import antenv
        stub = types.ModuleType("antenv.axon_hooks")
        stub.get_axon_ntff_profile_hook = lambda: None
        stub.set_axon_ntff_profile_hook = lambda h: None
        sys.modules["antenv.axon_hooks"] = stub
        antenv.axon_hooks = stub

    from concourse.bass_utils import run_bass_kernel_spmd
    res = run_bass_kernel_spmd(nc, in_maps, core_ids=list(range(_N_CORES)))
    globals()["_LAST_RESULT"] = res

    out = np.empty((B, _C, T, F), np.float32)
    for m in range(_N_CORES):
        ym = res.results[m]["y"].astype(np.float32)     # (TCH, P, 2050)
        ym = ym.reshape(B, TS, F, _C)                   # tok=(b, t_local)
        out[:, :, m * TS:(m + 1) * TS, :] = ym.transpose(0, 3, 1, 2)
    return out


# revision 22
# speedup vs baseline: 1.0896x; 1.0896x over previous
"""BandSplit (gather -> per-band MLP -> scatter-add OLA -> /ola) on 8 TRN2 cores.

Strategy
--------
The whole reference computation is linear in x, so on the host we fold the
per-band pre/post weights, melbank weights, mask, scatter-add and /ola into a
single banded matrix A of shape (C*F, C*F) mapping the (c, f) spectrum of one
(b, t) token to the output spectrum (see _fold_matrix).  The device kernel is
a banded matmul, data-parallel over the 4096 (b, t) tokens across the 8
NeuronCores (512 tokens/core, 4 chunks of 128) with zero cross-core traffic.

v2 layout (vs the v0 baseline):
 * Host pre-transposes x into contraction-major layout (partition = 64
   consecutive f values x 2 input channels), so the PE transposes and the
   gpsimd cast-DMAs are gone entirely and both input channels contract in a
   single matmul pass (halves the number of PE passes; 64-row windows are
   narrower than two 128-row passes: 4764 vs 6956 cols per token chunk).
 * x for low-frequency groups 0..13 ships as fp8 e3m4 (x2 pre-scale folded
   into A) to cut HBM read traffic; high groups stay bf16 where the wide
   bands accumulate too many terms for fp8 (measured rel-err 0.011 vs the
   2e-2 budget).  A stays bf16.
 * All loads are fat HWDGE DMAs on the sync ring; stores go on the scalar
   ring so they never head-of-line block loads.
 * PSUM holds one token-chunk of output (2050 interleaved f32 cols, 5 banks);
   drains are bank-granular, alternating DVE/ACT, so the next chunk's
   matmuls only wait for the one bank they touch.
"""

import numpy as np

_P = 128
_G = 64            # f rows per partition group (x2 channels = 128 partitions)
_C = 2
_F = 1025
_NG = 17           # groups cover f = 0..1087 (1025 real + bias row 1025)
_FP8_GROUPS = 14   # groups 0..13 in e3m4, 14..16 in bf16
_FP8_SCALE = 2.0
_TOK_CORE = 512    # tokens per core
_TCH = 4           # token chunks of 128
_N_CORES = 8


def _fold_matrix(pre_w, pre_b, post_w, post_b, idx, melw, mask, ola_window):
    """Fold the full reference computation into (A, const).

    A: (C, F, C, F) with out[co, fo] = sum_{ci, fi} x[ci, fi] * A[ci, fi, co, fo]
    const: (C, F) additive constant from the biases.
    """
    K, W = idx.shape
    C = _C
    F = ola_window.shape[0]

    pre_w = np.asarray(pre_w, np.float64)
    post_w = np.asarray(post_w, np.float64)
    pre_b = np.asarray(pre_b, np.float64)
    post_b = np.asarray(post_b, np.float64)
    wts = (np.asarray(melw, np.float64) * np.asarray(mask, np.float64))
    msk = np.asarray(mask, np.float64)
    idx = np.asarray(idx)

    M = np.einsum('kio,koj->kij', pre_w, post_w).reshape(K, W, C, W, C)
    vals = M * wts[:, :, None, None, None] * msk[:, None, None, :, None]

    fin = idx[:, :, None, None, None].astype(np.int64)
    fout = idx[:, None, None, :, None].astype(np.int64)
    cin = np.arange(C)[None, None, :, None, None]
    cout = np.arange(C)[None, None, None, None, :]
    flat = ((cin * F + fin) * C + cout) * F + fout
    A = np.bincount(
        np.broadcast_to(flat, vals.shape).ravel(), weights=vals.ravel(),
        minlength=C * F * C * F,
    ).reshape(C, F, C, F)
    A /= ola_window[None, None, None, :]

    bv = (np.einsum('ko,koj->kj', pre_b, post_w) + post_b).reshape(K, W, C)
    bv = bv * msk[:, :, None]
    cflat = (np.arange(C)[None, None, :] * F + idx[:, :, None]).astype(np.int64)
    const = np.bincount(
        np.broadcast_to(cflat, bv.shape).ravel(), weights=bv.ravel(),
        minlength=C * F,
    ).reshape(C, F)
    const /= ola_window[None, :]
    return A, const


def _plan(A, const):
    """Build the banded layout: per-group windows, packed A, segment lists.

    Rows of group j (128 partitions): p = 2*(f - 64j) + ci for f in
    [64j, 64j+64), both channels.  f == 1025 is the bias row (x column == 1).
    Output columns are channel-interleaved: col = 2*fo + co, 2050 total.
    """
    F, C, NG, G = _F, _C, _NG, _G
    # Ap[ci, f, co, fo] over padded f rows (F+1 rows: bias at F)
    Ap = np.zeros((C, NG * G, C, F), np.float64)
    Ap[:, :F] = A
    Ap[0, F] = const

    nzrow = (Ap != 0).any(axis=(0, 2))          # (NG*G, F) over (f, fo)
    wins = []
    for j in range(NG):
        cols = nzrow[j * G:(j + 1) * G].any(axis=0)
        nzc = np.nonzero(cols)[0]
        lo, hi = (int(nzc[0]), int(nzc[-1]) + 1) if len(nzc) else (0, 1)
        wins.append((lo, hi))
    cov = np.zeros(F, bool)
    for lo, hi in wins:
        cov[lo:hi] = True
    assert cov.all(), 'window coverage hole'

    # packed A: [128, TW] with 16-col-aligned per-group blocks
    offs, tw = [], 0
    for j in range(NG):
        offs.append(tw)
        tw += (2 * (wins[j][1] - wins[j][0]) + 15) // 16 * 16
    import ml_dtypes
    ab = np.zeros((_P, tw), ml_dtypes.bfloat16)
    for j in range(NG):
        lo, hi = wins[j]
        blk = Ap[:, j * G:(j + 1) * G, :, lo:hi]       # (ci, 64, co, w)
        blk = blk.transpose(1, 0, 3, 2).reshape(_P, -1)  # p=(f,ci), col=(fo,co)
        if j < _FP8_GROUPS:
            blk = blk / _FP8_SCALE                     # undo x pre-scale
        ab[:, offs[j]:offs[j] + 2 * (hi - lo)] = blk

    # matmul segments per group, split at 512-col PSUM bank boundaries
    segs = []                                          # [(j, s, e)] in order
    for j in range(NG):
        lo2, hi2 = 2 * wins[j][0], 2 * wins[j][1]
        s = lo2
        while s < hi2:
            e = min(hi2, (s // 512 + 1) * 512)
            segs.append((j, s, e))
            s = e
    # first/last toucher of each bank (for start/stop flags), per chunk
    bank_first, bank_last = {}, {}
    for i, (j, s, e) in enumerate(segs):
        b = s // 512
        bank_first.setdefault(b, i)
        bank_last[b] = i
    return wins, offs, tw, ab, segs, bank_first, bank_last


_PROGRAM_CACHE = {}


def _build_program(tw, wins, offs, segs, bank_first, bank_last):
    import concourse.bass as bass  # noqa: F401
    import concourse.tile as tile
    import concourse.mybir as mybir
    from concourse import bacc
    from concourse.masks import make_identity

    f32 = mybir.dt.float32
    bf16 = mybir.dt.bfloat16
    f16 = mybir.dt.float16
    fp8 = mybir.dt.float8e3
    P = _P
    NG, TCH = _NG, _TCH
    N8 = _FP8_GROUPS
    N16 = NG - N8
    W_OUT = 2 * _F                      # 2050 interleaved output cols

    nc = bacc.Bacc("TRN2", target_bir_lowering=False, debug=False,
                   num_devices=_N_CORES)
    xs8 = nc.dram_tensor("xs8", [P, N8 * _TOK_CORE], fp8, kind="ExternalInput")
    xs16 = nc.dram_tensor("xs16", [P, N16 * _TOK_CORE], bf16,
                          kind="ExternalInput")
    ab = nc.dram_tensor("ab", [P, tw], bf16, kind="ExternalInput")
    y = nc.dram_tensor("y", [TCH, P, W_OUT], f16, kind="ExternalOutput")

    # loads stream on three DMA rings concurrently, byte-balanced (sync:
    # x fp8; scalar: A groups 0-11; gpsimd: A groups 12-16 then x bf16);
    # stores alternate between the sync and gpsimd rings
    X8_SPLITS = [(0, 2), (2, 5), (5, 8), (8, 11), (11, 14)]
    A_SPLITS_SC = [(0, 3), (3, 6), (6, 9), (9, 12)]
    A_SPLITS_GP = [(12, 15), (15, 17)]
    NBANK = (W_OUT + 511) // 512            # 5 PSUM banks (last holds 2 cols)

    with tile.TileContext(nc) as tc:
        with (
            tc.tile_pool(name="xpool", bufs=1) as xpool,
            tc.tile_pool(name="apool", bufs=1) as apool,
            tc.tile_pool(name="opool", bufs=4) as opool,
            tc.tile_pool(name="idpool", bufs=1) as idpool,
            tc.tile_pool(name="psa", bufs=1, space="PSUM") as psa,
            tc.tile_pool(name="psb", bufs=2, space="PSUM") as psb,
        ):
            x8t = xpool.tile([P, N8 * _TOK_CORE], fp8, name="x8")
            x16t = xpool.tile([P, N16 * _TOK_CORE], bf16, name="x16")
            abt = apool.tile([P, tw], bf16, name="abt")
            ident = idpool.tile([P, P], bf16, name="ident")
            make_identity(nc, ident[:])

            S = _TOK_CORE
            for g0, g1 in X8_SPLITS:
                nc.sync.dma_start(x8t[:, g0 * S:g1 * S], xs8[:, g0 * S:g1 * S])
            for a0, a1 in A_SPLITS_SC:
                nc.scalar.dma_start(abt[:, offs[a0]:offs[a1]],
                                    ab[:, offs[a0]:offs[a1]])
            for a0, a1 in A_SPLITS_GP:
                o1 = tw if a1 >= NG else offs[a1]
                nc.gpsimd.dma_start(abt[:, offs[a0]:o1], ab[:, offs[a0]:o1])
            nc.gpsimd.dma_start(x16t[:], xs16[:])

            # PE warmup: >=3.4us of continuous matmuls trips the HAM clock
            # gate to 2.4 GHz while the DMAs land
            warm = psa.tile([P, P], f32, tag="warm", name="warm")
            for _ in range(36):
                nc.tensor.matmul(warm[:], ident[:], ident[:],
                                 start=True, stop=True)

            def lhsT(j, t):
                if j < N8:
                    return x8t[:, j * S + t * P:j * S + (t + 1) * P]
                return x16t[:, (j - N8) * S + t * P:(j - N8) * S + (t + 1) * P]

            # one PSUM tile per 512-col bank: drain deps are bank-granular.
            # Banks 3/4 finish at the very end of each chunk, so they get
            # double buffers; banks 0-2 drain mid-chunk and single-buffer.
            def bank_tile(t, b):
                w = min(512, W_OUT - b * 512)
                pool = psb if b >= 3 else psa
                return pool.tile([P, w], f32, tag=f"ptb{b}",
                                 name=f"pt_{t}_{b}")

            segs_by_group = {}
            for i, (j, s, e) in enumerate(segs):
                segs_by_group.setdefault(j, []).append((i, s, e))
            bank_total = {}
            for (j, s, e) in segs:
                bank_total[s // 512] = bank_total.get(s // 512, 0) + 1

            # interleaved chunk schedule: while chunk 0's tail groups wait on
            # the input DMAs, run chunk 1/2's early groups whose PSUM banks
            # chunk 0 has already retired (bank b of chunk t may start only
            # after chunk t-1's bank b drained: b0 retires at g4, b1 at g8,
            # b2 at g13, b3/b4 are double-buffered).
            SCHED = [(0, 0, 12), (1, 0, 7), (2, 0, 3), (0, 12, NG),
                     (1, 7, NG), (2, 3, NG), (3, 0, NG)]
            assert sorted((t, j) for t, j0, j1 in SCHED
                          for j in range(j0, j1)) == \
                sorted((t, j) for t in range(TCH) for j in range(NG))
            # static safety: chunk t's first touch of bank b must come after
            # the bank's previous user fully retired (PE FIFO deadlocks
            # otherwise, since the drain it waits on would be emitted later)
            emit_order = [(t, s // 512) for (t, j0, j1) in SCHED
                          for j in range(j0, j1)
                          for (i, s, e) in segs_by_group[j]]
            first_touch, retire_pos, cnt = {}, {}, {}
            for pos, (t, b) in enumerate(emit_order):
                first_touch.setdefault((t, b), pos)
                cnt[(t, b)] = cnt.get((t, b), 0) + 1
                if cnt[(t, b)] == bank_total[b]:
                    retire_pos[(t, b)] = pos
            nbufs = {0: 1, 1: 1, 2: 1, 3: 2, 4: 2}
            for (t, b), pos in first_touch.items():
                if t >= nbufs[b]:
                    assert retire_pos[(t - nbufs[b], b)] < pos, (t, b)

            pts, ots = {}, {}
            bank_done = {}
            drained = {t: set() for t in range(TCH)}
            drain_eng = {0: 'v', 1: 's', 2: 'v', 3: 's', 4: 'v'}

            # stores spread across the three rings so no single ring's
            # backlog delays an ot-buffer reuse or drain
            store_ring0 = {0: nc.sync, 1: nc.gpsimd, 2: nc.sync, 3: nc.gpsimd}
            store_ring1 = {0: nc.scalar, 1: nc.scalar, 2: nc.gpsimd,
                           3: nc.scalar}

            nsched = 0
            for (t, j0, j1) in SCHED:
                for j in range(j0, j1):
                    # two absorbed warm matmuls ahead of each load-gated
                    # group keep PE duty high enough for the HAM clock gate
                    if nsched < 22:
                        nsched += 1
                        for _ in range(2):
                            nc.tensor.matmul(warm[:], ident[:], ident[:],
                                             start=True, stop=True)
                    lo2 = 2 * wins[j][0]
                    o = offs[j]
                    for (i, s, e) in segs_by_group[j]:
                        b = s // 512
                        if (t, b) not in pts:
                            pts[(t, b)] = bank_tile(t, b)
                        nc.tensor.matmul(
                            pts[(t, b)][:, s - b * 512:e - b * 512],
                            lhsT(j, t),
                            abt[:, o + s - lo2:o + e - lo2],
                            start=(bank_first[b] == i),
                            stop=(bank_last[b] == i),
                        )
                        bank_done[(t, b)] = bank_done.get((t, b), 0) + 1
                        if bank_done[(t, b)] < bank_total[b]:
                            continue
                        # bank (t, b) retired: drain it now, store halves as
                        # soon as their banks are all in SBUF
                        if t not in ots:
                            ots[t] = opool.tile([P, W_OUT], f16, tag="out",
                                                name=f"out_{t}")
                        dst = ots[t][:, b * 512:b * 512 + min(
                            512, W_OUT - b * 512)]
                        if drain_eng[b] == 'v':
                            nc.vector.tensor_copy(dst, pts[(t, b)][:])
                        else:
                            nc.scalar.copy(dst, pts[(t, b)][:])
                        drained[t].add(b)
                        if b in (0, 1) and {0, 1} <= drained[t]:
                            store_ring0[t].dma_start(y[t, :, 0:1024],
                                                     ots[t][:, 0:1024])
                        if {2, 3, 4} <= drained[t]:
                            store_ring1[t].dma_start(y[t, :, 1024:W_OUT],
                                                     ots[t][:, 1024:W_OUT])

    nc.compile()
    return nc


def kernel(**inputs):
    import ml_dtypes

    x = np.ascontiguousarray(np.asarray(inputs["x"], np.float32))
    B, C, T, F = x.shape
    assert (B, C, F) == (4, _C, _F), (B, C, F)
    TS = T // _N_CORES

    A, const = _fold_matrix(
        inputs["pre_w"], inputs["pre_b"], inputs["post_w"], inputs["post_b"],
        inputs["idx"], inputs["melw"], inputs["mask"], inputs["ola_window"],
    )
    wins, offs, tw, ab, segs, bank_first, bank_last = _plan(A, const)

    key = (tw, tuple(wins))
    if key not in _PROGRAM_CACHE:
        _PROGRAM_CACHE[key] = _build_program(tw, wins, offs, segs,
                                             bank_first, bank_last)
    nc = _PROGRAM_CACHE[key]

    # host pre-shard: contraction-major x layout per core.
    # xq[ci, f, b, t] with f padded to 1088 (bias row at f=1025 == 1.0)
    NGG = _NG * _G
    xq = np.zeros((_C, NGG, B, T), np.float32)
    xq[:, :F] = x.transpose(1, 3, 0, 2)
    xq[0, F] = 1.0
    # [NG, G, C, B, T] -> partitions p = 2*f_off + ci
    xq = xq.reshape(_C, _NG, _G, B, T).transpose(1, 2, 0, 3, 4)
    x8 = (xq[:_FP8_GROUPS] * _FP8_SCALE).astype(ml_dtypes.float8_e3m4)
    x16 = xq[_FP8_GROUPS:].astype(ml_dtypes.bfloat16)

    in_maps = []
    for m in range(_N_CORES):
        sl8 = x8[:, :, :, :, m * TS:(m + 1) * TS]      # (N8, G, C, B, TS)
        sl16 = x16[:, :, :, :, m * TS:(m + 1) * TS]
        in_maps.append({
            "xs8": np.ascontiguousarray(
                sl8.reshape(_FP8_GROUPS, _P // 2 // 1, _C, _TOK_CORE)
                   .reshape(_FP8_GROUPS, _G * _C, _TOK_CORE)
                   .transpose(1, 0, 2).reshape(_P, -1)),
            "xs16": np.ascontiguousarray(
                sl16.reshape(_NG - _FP8_GROUPS, _G * _C, _TOK_CORE)
                    .transpose(1, 0, 2).reshape(_P, -1)),
            "ab": ab,
        })

    try:
        import antenv.axon_hooks  # noqa: F401
    except ImportError:
        import sys
        import types
        import antenv
        stub = types.ModuleType("antenv.axon_hooks")
        stub.get_axon_ntff_profile_hook = lambda: None
        stub.set_axon_ntff_profile_hook = lambda h: None
        sys.modules["antenv.axon_hooks"] = stub
        antenv.axon_hooks = stub

    from concourse.bass_utils import run_bass_kernel_spmd
    res = run_bass_kernel_spmd(nc, in_maps, core_ids=list(range(_N_CORES)))
    globals()["_LAST_RESULT"] = res

    out = np.empty((B, _C, T, F), np.float32)
    for m in range(_N_CORES):
        ym = res.results[m]["y"].astype(np.float32)     # (TCH, P, 2050)
        ym = ym.reshape(B, TS, F, _C)                   # tok=(b, t_local)
        out[:, :, m * TS:(m + 1) * TS, :] = ym.transpose(0, 3, 1, 2)
    return out


# revision 27
# speedup vs baseline: 1.1312x; 1.0381x over previous
"""BandSplit (gather -> per-band MLP -> scatter-add OLA -> /ola) on 8 TRN2 cores.

Strategy
--------
The whole reference computation is linear in x, so on the host we fold the
per-band pre/post weights, melbank weights, mask, scatter-add and /ola into a
single banded matrix A of shape (C*F, C*F) mapping the (c, f) spectrum of one
(b, t) token to the output spectrum (see _fold_matrix).  The device kernel is
a banded matmul, data-parallel over the 4096 (b, t) tokens across the 8
NeuronCores (512 tokens/core, 4 chunks of 128) with zero cross-core traffic.

v2 layout (vs the v0 baseline):
 * Host pre-transposes x into contraction-major layout (partition = 64
   consecutive f values x 2 input channels), so the PE transposes and the
   gpsimd cast-DMAs are gone entirely and both input channels contract in a
   single matmul pass (halves the number of PE passes; 64-row windows are
   narrower than two 128-row passes: 4764 vs 6956 cols per token chunk).
 * x for low-frequency groups 0..13 ships as fp8 e3m4 (x2 pre-scale folded
   into A) to cut HBM read traffic; high groups stay bf16 where the wide
   bands accumulate too many terms for fp8 (measured rel-err 0.011 vs the
   2e-2 budget).  A stays bf16.
 * All loads are fat HWDGE DMAs on the sync ring; stores go on the scalar
   ring so they never head-of-line block loads.
 * PSUM holds one token-chunk of output (2050 interleaved f32 cols, 5 banks);
   drains are bank-granular, alternating DVE/ACT, so the next chunk's
   matmuls only wait for the one bank they touch.
"""

import numpy as np

_P = 128
_G = 64            # f rows per partition group (x2 channels = 128 partitions)
_C = 2
_F = 1025
_NG = 17           # groups cover f = 0..1087 (1025 real + bias row 1025)
_FP8_GROUPS = 14   # groups 0..13 in e3m4, 14..16 in bf16
_FP8_SCALE = 2.0
_TOK_CORE = 512    # tokens per core
_TCH = 4           # token chunks of 128
_N_CORES = 8


def _fold_matrix(pre_w, pre_b, post_w, post_b, idx, melw, mask, ola_window):
    """Fold the full reference computation into (A, const).

    A: (C, F, C, F) with out[co, fo] = sum_{ci, fi} x[ci, fi] * A[ci, fi, co, fo]
    const: (C, F) additive constant from the biases.
    """
    K, W = idx.shape
    C = _C
    F = ola_window.shape[0]

    pre_w = np.asarray(pre_w, np.float64)
    post_w = np.asarray(post_w, np.float64)
    pre_b = np.asarray(pre_b, np.float64)
    post_b = np.asarray(post_b, np.float64)
    wts = (np.asarray(melw, np.float64) * np.asarray(mask, np.float64))
    msk = np.asarray(mask, np.float64)
    idx = np.asarray(idx)

    M = np.einsum('kio,koj->kij', pre_w, post_w).reshape(K, W, C, W, C)
    vals = M * wts[:, :, None, None, None] * msk[:, None, None, :, None]

    fin = idx[:, :, None, None, None].astype(np.int64)
    fout = idx[:, None, None, :, None].astype(np.int64)
    cin = np.arange(C)[None, None, :, None, None]
    cout = np.arange(C)[None, None, None, None, :]
    flat = ((cin * F + fin) * C + cout) * F + fout
    A = np.bincount(
        np.broadcast_to(flat, vals.shape).ravel(), weights=vals.ravel(),
        minlength=C * F * C * F,
    ).reshape(C, F, C, F)
    A /= ola_window[None, None, None, :]

    bv = (np.einsum('ko,koj->kj', pre_b, post_w) + post_b).reshape(K, W, C)
    bv = bv * msk[:, :, None]
    cflat = (np.arange(C)[None, None, :] * F + idx[:, :, None]).astype(np.int64)
    const = np.bincount(
        np.broadcast_to(cflat, bv.shape).ravel(), weights=bv.ravel(),
        minlength=C * F,
    ).reshape(C, F)
    const /= ola_window[None, :]
    return A, const


def _plan(A, const):
    """Build the banded layout: per-group windows, packed A, segment lists.

    Rows of group j (128 partitions): p = 2*(f - 64j) + ci for f in
    [64j, 64j+64), both channels.  f == 1025 is the bias row (x column == 1).
    Output columns are channel-interleaved: col = 2*fo + co, 2050 total.
    """
    F, C, NG, G = _F, _C, _NG, _G
    # Ap[ci, f, co, fo] over padded f rows (F+1 rows: bias at F)
    Ap = np.zeros((C, NG * G, C, F), np.float64)
    Ap[:, :F] = A
    Ap[0, F] = const

    nzrow = (Ap != 0).any(axis=(0, 2))          # (NG*G, F) over (f, fo)
    wins = []
    for j in range(NG):
        cols = nzrow[j * G:(j + 1) * G].any(axis=0)
        nzc = np.nonzero(cols)[0]
        lo, hi = (int(nzc[0]), int(nzc[-1]) + 1) if len(nzc) else (0, 1)
        wins.append((lo, hi))
    cov = np.zeros(F, bool)
    for lo, hi in wins:
        cov[lo:hi] = True
    assert cov.all(), 'window coverage hole'

    # packed A: [128, TW] with 16-col-aligned per-group blocks
    offs, tw = [], 0
    for j in range(NG):
        offs.append(tw)
        tw += (2 * (wins[j][1] - wins[j][0]) + 15) // 16 * 16
    import ml_dtypes
    ab = np.zeros((_P, tw), ml_dtypes.bfloat16)
    for j in range(NG):
        lo, hi = wins[j]
        blk = Ap[:, j * G:(j + 1) * G, :, lo:hi]       # (ci, 64, co, w)
        blk = blk.transpose(1, 0, 3, 2).reshape(_P, -1)  # p=(f,ci), col=(fo,co)
        if j < _FP8_GROUPS:
            blk = blk / _FP8_SCALE                     # undo x pre-scale
        ab[:, offs[j]:offs[j] + 2 * (hi - lo)] = blk

    # matmul segments per group, split at 512-col PSUM bank boundaries
    segs = []                                          # [(j, s, e)] in order
    for j in range(NG):
        lo2, hi2 = 2 * wins[j][0], 2 * wins[j][1]
        s = lo2
        while s < hi2:
            e = min(hi2, (s // 512 + 1) * 512)
            segs.append((j, s, e))
            s = e
    # first/last toucher of each bank (for start/stop flags), per chunk
    bank_first, bank_last = {}, {}
    for i, (j, s, e) in enumerate(segs):
        b = s // 512
        bank_first.setdefault(b, i)
        bank_last[b] = i
    return wins, offs, tw, ab, segs, bank_first, bank_last


_PROGRAM_CACHE = {}


def _build_program(tw, wins, offs, segs, bank_first, bank_last):
    import concourse.bass as bass  # noqa: F401
    import concourse.tile as tile
    import concourse.mybir as mybir
    from concourse import bacc
    from concourse.masks import make_identity

    f32 = mybir.dt.float32
    bf16 = mybir.dt.bfloat16
    f16 = mybir.dt.float16
    fp8 = mybir.dt.float8e3
    P = _P
    NG, TCH = _NG, _TCH
    N8 = _FP8_GROUPS
    N16 = NG - N8
    W_OUT = 2 * _F                      # 2050 interleaved output cols

    nc = bacc.Bacc("TRN2", target_bir_lowering=False, debug=False,
                   num_devices=_N_CORES)
    # x ships chunk-major: one fat-line DMA per token chunk so every chunk's
    # x is resident early and both other rings are free to stream A, which
    # is what actually paces the chunk-0 matmuls
    xs8 = nc.dram_tensor("xs8", [TCH, P, N8 * P], fp8, kind="ExternalInput")
    xs16 = nc.dram_tensor("xs16", [TCH, P, N16 * P], bf16,
                          kind="ExternalInput")
    ab = nc.dram_tensor("ab", [P, tw], bf16, kind="ExternalInput")
    y = nc.dram_tensor("y", [TCH, P, W_OUT], f16, kind="ExternalOutput")

    A_SPLITS_SC = [(0, 3), (3, 6), (6, 9)]          # scalar ring
    A_SPLITS_GP = [(9, 12), (12, 15), (15, 17)]     # gpsimd ring
    NBANK = (W_OUT + 511) // 512            # 5 PSUM banks (last holds 2 cols)

    with tile.TileContext(nc) as tc:
        with (
            tc.tile_pool(name="xpool", bufs=1) as xpool,
            tc.tile_pool(name="apool", bufs=1) as apool,
            tc.tile_pool(name="opool", bufs=4) as opool,
            tc.tile_pool(name="idpool", bufs=1) as idpool,
            tc.tile_pool(name="psa", bufs=1, space="PSUM") as psa,
            tc.tile_pool(name="psb", bufs=2, space="PSUM") as psb,
        ):
            S8 = N8 * P
            S16 = N16 * P
            x8t = xpool.tile([P, TCH * S8], fp8, name="x8")
            x16t = xpool.tile([P, TCH * S16], bf16, name="x16")
            abt = apool.tile([P, tw], bf16, name="abt")
            ident = idpool.tile([P, P], bf16, name="ident")
            make_identity(nc, ident[:])

            # sync ring: all x, chunk-major; scalar + gpsimd: A in parallel
            for t in range(TCH):
                nc.sync.dma_start(x8t[:, t * S8:(t + 1) * S8], xs8[t])
                nc.sync.dma_start(x16t[:, t * S16:(t + 1) * S16], xs16[t])
            for a0, a1 in A_SPLITS_SC:
                nc.scalar.dma_start(abt[:, offs[a0]:offs[a1]],
                                    ab[:, offs[a0]:offs[a1]])
            for a0, a1 in A_SPLITS_GP:
                o1 = tw if a1 >= NG else offs[a1]
                nc.gpsimd.dma_start(abt[:, offs[a0]:o1], ab[:, offs[a0]:o1])

            # PE warmup: >=3.4us of continuous matmuls trips the HAM clock
            # gate to 2.4 GHz while the DMAs land
            warm = psa.tile([P, P], f32, tag="warm", name="warm")
            for _ in range(36):
                nc.tensor.matmul(warm[:], ident[:], ident[:],
                                 start=True, stop=True)

            def lhsT(j, t):
                if j < N8:
                    return x8t[:, t * S8 + j * P:t * S8 + (j + 1) * P]
                jj = j - N8
                return x16t[:, t * S16 + jj * P:t * S16 + (jj + 1) * P]

            # one PSUM tile per 512-col bank: drain deps are bank-granular.
            # Banks 3/4 finish at the very end of each chunk, so they get
            # double buffers; banks 0-2 drain mid-chunk and single-buffer.
            def bank_tile(t, b):
                w = min(512, W_OUT - b * 512)
                pool = psb if b >= 3 else psa
                return pool.tile([P, w], f32, tag=f"ptb{b}",
                                 name=f"pt_{t}_{b}")

            segs_by_group = {}
            for i, (j, s, e) in enumerate(segs):
                segs_by_group.setdefault(j, []).append((i, s, e))
            bank_total = {}
            for (j, s, e) in segs:
                bank_total[s // 512] = bank_total.get(s // 512, 0) + 1

            # interleaved chunk schedule: while chunk 0's tail groups wait on
            # the input DMAs, run chunk 1/2's early groups whose PSUM banks
            # chunk 0 has already retired (bank b of chunk t may start only
            # after chunk t-1's bank b drained: b0 retires at g4, b1 at g8,
            # b2 at g13, b3/b4 are double-buffered).
            SCHED = [(0, 0, 12), (1, 0, 7), (2, 0, 3), (0, 12, NG),
                     (1, 7, NG), (2, 3, NG), (3, 0, NG)]
            assert sorted((t, j) for t, j0, j1 in SCHED
                          for j in range(j0, j1)) == \
                sorted((t, j) for t in range(TCH) for j in range(NG))
            # static safety: chunk t's first touch of bank b must come after
            # the bank's previous user fully retired (PE FIFO deadlocks
            # otherwise, since the drain it waits on would be emitted later)
            emit_order = [(t, s // 512) for (t, j0, j1) in SCHED
                          for j in range(j0, j1)
                          for (i, s, e) in segs_by_group[j]]
            first_touch, retire_pos, cnt = {}, {}, {}
            for pos, (t, b) in enumerate(emit_order):
                first_touch.setdefault((t, b), pos)
                cnt[(t, b)] = cnt.get((t, b), 0) + 1
                if cnt[(t, b)] == bank_total[b]:
                    retire_pos[(t, b)] = pos
            nbufs = {0: 1, 1: 1, 2: 1, 3: 2, 4: 2}
            for (t, b), pos in first_touch.items():
                if t >= nbufs[b]:
                    assert retire_pos[(t - nbufs[b], b)] < pos, (t, b)

            pts, ots = {}, {}
            bank_done = {}
            drained = {t: set() for t in range(TCH)}
            drain_eng = {0: 'v', 1: 's', 2: 'v', 3: 's', 4: 'v'}

            # stores spread across the three rings so no single ring's
            # backlog delays an ot-buffer reuse or drain
            store_ring0 = {0: nc.sync, 1: nc.gpsimd, 2: nc.sync, 3: nc.gpsimd}
            store_ring1 = {0: nc.scalar, 1: nc.scalar, 2: nc.gpsimd,
                           3: nc.scalar}

            nsched = 0
            for (t, j0, j1) in SCHED:
                for j in range(j0, j1):
                    # two absorbed warm matmuls ahead of each load-gated
                    # group keep PE duty high enough for the HAM clock gate
                    if nsched < 27:
                        nsched += 1
                        for _ in range(2):
                            nc.tensor.matmul(warm[:], ident[:], ident[:],
                                             start=True, stop=True)
                    lo2 = 2 * wins[j][0]
                    o = offs[j]
                    for (i, s, e) in segs_by_group[j]:
                        b = s // 512
                        if (t, b) not in pts:
                            pts[(t, b)] = bank_tile(t, b)
                        nc.tensor.matmul(
                            pts[(t, b)][:, s - b * 512:e - b * 512],
                            lhsT(j, t),
                            abt[:, o + s - lo2:o + e - lo2],
                            start=(bank_first[b] == i),
                            stop=(bank_last[b] == i),
                        )
                        bank_done[(t, b)] = bank_done.get((t, b), 0) + 1
                        if bank_done[(t, b)] < bank_total[b]:
                            continue
                        # bank (t, b) retired: drain it now, store halves as
                        # soon as their banks are all in SBUF
                        if t not in ots:
                            ots[t] = opool.tile([P, W_OUT], f16, tag="out",
                                                name=f"out_{t}")
                        dst = ots[t][:, b * 512:b * 512 + min(
                            512, W_OUT - b * 512)]
                        if drain_eng[b] == 'v':
                            nc.vector.tensor_copy(dst, pts[(t, b)][:])
                        else:
                            nc.scalar.copy(dst, pts[(t, b)][:])
                        drained[t].add(b)
                        if b in (0, 1) and {0, 1} <= drained[t]:
                            store_ring0[t].dma_start(y[t, :, 0:1024],
                                                     ots[t][:, 0:1024])
                        if {2, 3, 4} <= drained[t]:
                            store_ring1[t].dma_start(y[t, :, 1024:W_OUT],
                                                     ots[t][:, 1024:W_OUT])

    nc.compile()
    return nc


def kernel(**inputs):
    import ml_dtypes

    x = np.ascontiguousarray(np.asarray(inputs["x"], np.float32))
    B, C, T, F = x.shape
    assert (B, C, F) == (4, _C, _F), (B, C, F)
    TS = T // _N_CORES

    A, const = _fold_matrix(
        inputs["pre_w"], inputs["pre_b"], inputs["post_w"], inputs["post_b"],
        inputs["idx"], inputs["melw"], inputs["mask"], inputs["ola_window"],
    )
    wins, offs, tw, ab, segs, bank_first, bank_last = _plan(A, const)

    key = (tw, tuple(wins))
    if key not in _PROGRAM_CACHE:
        _PROGRAM_CACHE[key] = _build_program(tw, wins, offs, segs,
                                             bank_first, bank_last)
    nc = _PROGRAM_CACHE[key]

    # host pre-shard: contraction-major x layout per core.
    # xq[ci, f, b, t] with f padded to 1088 (bias row at f=1025 == 1.0)
    NGG = _NG * _G
    xq = np.zeros((_C, NGG, B, T), np.float32)
    xq[:, :F] = x.transpose(1, 3, 0, 2)
    xq[0, F] = 1.0
    # [NG, G, C, B, T] -> partitions p = 2*f_off + ci
    xq = xq.reshape(_C, _NG, _G, B, T).transpose(1, 2, 0, 3, 4)
    x8 = (xq[:_FP8_GROUPS] * _FP8_SCALE).astype(ml_dtypes.float8_e3m4)
    x16 = xq[_FP8_GROUPS:].astype(ml_dtypes.bfloat16)

    in_maps = []
    for m in range(_N_CORES):
        # chunk-major layout: xs[t, p, j*128 + tok_local], chunk t == batch b
        sl8 = x8[:, :, :, :, m * TS:(m + 1) * TS]      # (N8, G, C, B, TS)
        sl16 = x16[:, :, :, :, m * TS:(m + 1) * TS]
        in_maps.append({
            "xs8": np.ascontiguousarray(
                sl8.transpose(3, 1, 2, 0, 4)           # (B, G, C, N8, TS)
                   .reshape(_TCH, _P, _FP8_GROUPS * _P)),
            "xs16": np.ascontiguousarray(
                sl16.transpose(3, 1, 2, 0, 4)
                    .reshape(_TCH, _P, (_NG - _FP8_GROUPS) * _P)),
            "ab": ab,
        })

    try:
        import antenv.axon_hooks  # noqa: F401
    except ImportError:
        import sys
        import types
        import antenv
        stub = types.ModuleType("antenv.axon_hooks")
        stub.get_axon_ntff_profile_hook = lambda: None
        stub.set_axon_ntff_profile_hook = lambda h: None
        sys.modules["antenv.axon_hooks"] = stub
        antenv.axon_hooks = stub

    from concourse.bass_utils import run_bass_kernel_spmd
    res = run_bass_kernel_spmd(nc, in_maps, core_ids=list(range(_N_CORES)))
    globals()["_LAST_RESULT"] = res

    out = np.empty((B, _C, T, F), np.float32)
    for m in range(_N_CORES):
        ym = res.results[m]["y"].astype(np.float32)     # (TCH, P, 2050)
        ym = ym.reshape(B, TS, F, _C)                   # tok=(b, t_local)
        out[:, :, m * TS:(m + 1) * TS, :] = ym.transpose(0, 3, 1, 2)
    return out


# revision 34
# speedup vs baseline: 1.2293x; 1.0868x over previous
"""BandSplit (gather -> per-band MLP -> scatter-add OLA -> /ola) on 8 TRN2 cores.

Strategy
--------
The whole reference computation is linear in x, so on the host we fold the
per-band pre/post weights, melbank weights, mask, scatter-add and /ola into a
single banded matrix A of shape (C*F, C*F) mapping the (c, f) spectrum of one
(b, t) token to the output spectrum (see _fold_matrix).  The device kernel is
a banded matmul, data-parallel over the 4096 (b, t) tokens across the 8
NeuronCores (512 tokens/core, 4 chunks of 128) with zero cross-core traffic.

v2 layout (vs the v0 baseline):
 * Host pre-transposes x into contraction-major layout (partition = 64
   consecutive f values x 2 input channels), so the PE transposes and the
   gpsimd cast-DMAs are gone entirely and both input channels contract in a
   single matmul pass (halves the number of PE passes; 64-row windows are
   narrower than two 128-row passes: 4764 vs 6956 cols per token chunk).
 * x for low-frequency groups 0..13 ships as fp8 e3m4 (x2 pre-scale folded
   into A) to cut HBM read traffic; high groups stay bf16 where the wide
   bands accumulate too many terms for fp8 (measured rel-err 0.011 vs the
   2e-2 budget).  A stays bf16.
 * All loads are fat HWDGE DMAs on the sync ring; stores go on the scalar
   ring so they never head-of-line block loads.
 * PSUM holds one token-chunk of output (2050 interleaved f32 cols, 5 banks);
   drains are bank-granular, alternating DVE/ACT, so the next chunk's
   matmuls only wait for the one bank they touch.
"""

import numpy as np

_P = 128
_G = 64            # f rows per partition group (x2 channels = 128 partitions)
_C = 2
_F = 1025
_NG = 17           # groups cover f = 0..1087 (1025 real + bias row 1025)
_FP8_GROUPS = 14   # groups 0..13 in e3m4, 14..16 in bf16
_FP8_SCALE = 2.0
_TOK_CORE = 512    # tokens per core
_TCH = 4           # token chunks of 128
_N_CORES = 8


def _fold_matrix(pre_w, pre_b, post_w, post_b, idx, melw, mask, ola_window):
    """Fold the full reference computation into (A, const).

    A: (C, F, C, F) with out[co, fo] = sum_{ci, fi} x[ci, fi] * A[ci, fi, co, fo]
    const: (C, F) additive constant from the biases.
    """
    K, W = idx.shape
    C = _C
    F = ola_window.shape[0]

    pre_w = np.asarray(pre_w, np.float64)
    post_w = np.asarray(post_w, np.float64)
    pre_b = np.asarray(pre_b, np.float64)
    post_b = np.asarray(post_b, np.float64)
    wts = (np.asarray(melw, np.float64) * np.asarray(mask, np.float64))
    msk = np.asarray(mask, np.float64)
    idx = np.asarray(idx)

    M = np.einsum('kio,koj->kij', pre_w, post_w).reshape(K, W, C, W, C)
    vals = M * wts[:, :, None, None, None] * msk[:, None, None, :, None]

    fin = idx[:, :, None, None, None].astype(np.int64)
    fout = idx[:, None, None, :, None].astype(np.int64)
    cin = np.arange(C)[None, None, :, None, None]
    cout = np.arange(C)[None, None, None, None, :]
    flat = ((cin * F + fin) * C + cout) * F + fout
    A = np.bincount(
        np.broadcast_to(flat, vals.shape).ravel(), weights=vals.ravel(),
        minlength=C * F * C * F,
    ).reshape(C, F, C, F)
    A /= ola_window[None, None, None, :]

    bv = (np.einsum('ko,koj->kj', pre_b, post_w) + post_b).reshape(K, W, C)
    bv = bv * msk[:, :, None]
    cflat = (np.arange(C)[None, None, :] * F + idx[:, :, None]).astype(np.int64)
    const = np.bincount(
        np.broadcast_to(cflat, bv.shape).ravel(), weights=bv.ravel(),
        minlength=C * F,
    ).reshape(C, F)
    const /= ola_window[None, :]
    return A, const


def _plan(A, const):
    """Build the banded layout: per-group windows, packed A, segment lists.

    Rows of group j (128 partitions): p = 2*(f - 64j) + ci for f in
    [64j, 64j+64), both channels.  f == 1025 is the bias row (x column == 1).
    Output columns are channel-interleaved: col = 2*fo + co, 2050 total.
    """
    F, C, NG, G = _F, _C, _NG, _G
    # Ap[ci, f, co, fo] over padded f rows (F+1 rows: bias at F)
    Ap = np.zeros((C, NG * G, C, F), np.float64)
    Ap[:, :F] = A
    Ap[0, F] = const

    nzrow = (Ap != 0).any(axis=(0, 2))          # (NG*G, F) over (f, fo)
    wins = []
    for j in range(NG):
        cols = nzrow[j * G:(j + 1) * G].any(axis=0)
        nzc = np.nonzero(cols)[0]
        lo, hi = (int(nzc[0]), int(nzc[-1]) + 1) if len(nzc) else (0, 1)
        wins.append((lo, hi))
    cov = np.zeros(F, bool)
    for lo, hi in wins:
        cov[lo:hi] = True
    assert cov.all(), 'window coverage hole'

    # packed A: [128, TW] with 16-col-aligned per-group blocks
    offs, tw = [], 0
    for j in range(NG):
        offs.append(tw)
        tw += (2 * (wins[j][1] - wins[j][0]) + 15) // 16 * 16
    import ml_dtypes
    ab = np.zeros((_P, tw), ml_dtypes.bfloat16)
    for j in range(NG):
        lo, hi = wins[j]
        blk = Ap[:, j * G:(j + 1) * G, :, lo:hi]       # (ci, 64, co, w)
        blk = blk.transpose(1, 0, 3, 2).reshape(_P, -1)  # p=(f,ci), col=(fo,co)
        if j < _FP8_GROUPS:
            blk = blk / _FP8_SCALE                     # undo x pre-scale
        ab[:, offs[j]:offs[j] + 2 * (hi - lo)] = blk

    # matmul segments per group, split at 512-col PSUM bank boundaries
    segs = []                                          # [(j, s, e)] in order
    for j in range(NG):
        lo2, hi2 = 2 * wins[j][0], 2 * wins[j][1]
        s = lo2
        while s < hi2:
            e = min(hi2, (s // 512 + 1) * 512)
            segs.append((j, s, e))
            s = e
    # first/last toucher of each bank (for start/stop flags), per chunk
    bank_first, bank_last = {}, {}
    for i, (j, s, e) in enumerate(segs):
        b = s // 512
        bank_first.setdefault(b, i)
        bank_last[b] = i
    return wins, offs, tw, ab, segs, bank_first, bank_last


_PROGRAM_CACHE = {}


def _build_program(tw, wins, offs, segs, bank_first, bank_last):
    import concourse.bass as bass  # noqa: F401
    import concourse.tile as tile
    import concourse.mybir as mybir
    from concourse import bacc
    from concourse.masks import make_identity

    f32 = mybir.dt.float32
    bf16 = mybir.dt.bfloat16
    f16 = mybir.dt.float16
    fp8 = mybir.dt.float8e3
    P = _P
    NG, TCH = _NG, _TCH
    N8 = _FP8_GROUPS
    N16 = NG - N8
    W_OUT = 2 * _F                      # 2050 interleaved output cols

    nc = bacc.Bacc("TRN2", target_bir_lowering=False, debug=False,
                   num_devices=_N_CORES)
    # x ships chunk-major: one fat-line DMA per token chunk so every chunk's
    # x is resident early and both other rings are free to stream A, which
    # is what actually paces the chunk-0 matmuls
    xs8 = nc.dram_tensor("xs8", [TCH, P, N8 * P], fp8, kind="ExternalInput")
    xs16 = nc.dram_tensor("xs16", [TCH, P, N16 * P], bf16,
                          kind="ExternalInput")
    ab = nc.dram_tensor("ab", [P, tw], bf16, kind="ExternalInput")
    y = nc.dram_tensor("y", [TCH, P, W_OUT], f16, kind="ExternalOutput")

    # A splits alternate between the two rings so arrival order matches the
    # group-order consumption of the chunk-0 matmul stream
    A_SPLITS_SC = [(0, 3), (6, 9), (12, 15)]        # scalar ring
    A_SPLITS_GP = [(3, 6), (9, 12), (15, 17)]       # gpsimd ring
    NBANK = (W_OUT + 511) // 512            # 5 PSUM banks (last holds 2 cols)

    with tile.TileContext(nc) as tc:
        with (
            tc.tile_pool(name="xpool", bufs=1) as xpool,
            tc.tile_pool(name="apool", bufs=1) as apool,
            tc.tile_pool(name="opool", bufs=4) as opool,
            tc.tile_pool(name="idpool", bufs=1) as idpool,
            tc.tile_pool(name="psa", bufs=1, space="PSUM") as psa,
            tc.tile_pool(name="psb", bufs=2, space="PSUM") as psb,
        ):
            S8 = N8 * P
            S16 = N16 * P
            x8t = xpool.tile([P, TCH * S8], fp8, name="x8")
            x16t = xpool.tile([P, TCH * S16], bf16, name="x16")
            abt = apool.tile([P, tw], bf16, name="abt")
            ident = idpool.tile([P, P], bf16, name="ident")
            make_identity(nc, ident[:])

            # sync ring: all x, chunk-major; scalar + gpsimd: A in parallel
            for t in range(TCH):
                nc.sync.dma_start(x8t[:, t * S8:(t + 1) * S8], xs8[t])
                nc.sync.dma_start(x16t[:, t * S16:(t + 1) * S16], xs16[t])
            for a0, a1 in A_SPLITS_SC:
                nc.scalar.dma_start(abt[:, offs[a0]:offs[a1]],
                                    ab[:, offs[a0]:offs[a1]])
            for a0, a1 in A_SPLITS_GP:
                o1 = tw if a1 >= NG else offs[a1]
                nc.gpsimd.dma_start(abt[:, offs[a0]:o1], ab[:, offs[a0]:o1])

            # PE warmup: >=3.4us of continuous matmuls trips the HAM clock
            # gate to 2.4 GHz while the DMAs land.  The warm tile is the
            # first rotation of the double-buffered bank-2 tag, so it costs
            # no extra PSUM bank (t1's b2 reuses it, trivially WAR-safe).
            warm = psb.tile([P, 512], f32, tag="ptb2", name="warm")
            for _ in range(28):
                nc.tensor.matmul(warm[:, 0:P], ident[:], ident[:],
                                 start=True, stop=True)

            def lhsT(j, t):
                if j < N8:
                    return x8t[:, t * S8 + j * P:t * S8 + (j + 1) * P]
                jj = j - N8
                return x16t[:, t * S16 + jj * P:t * S16 + (jj + 1) * P]

            # one PSUM tile per 512-col bank: drain deps are bank-granular.
            # Banks 3/4 finish at the very end of each chunk, so they get
            # double buffers; banks 0-2 drain mid-chunk and single-buffer.
            def bank_tile(t, b):
                w = min(512, W_OUT - b * 512)
                pool = psb if b >= 2 else psa
                return pool.tile([P, w], f32, tag=f"ptb{b}",
                                 name=f"pt_{t}_{b}")

            segs_by_group = {}
            for i, (j, s, e) in enumerate(segs):
                segs_by_group.setdefault(j, []).append((i, s, e))
            bank_total = {}
            for (j, s, e) in segs:
                bank_total[s // 512] = bank_total.get(s // 512, 0) + 1

            # interleaved chunk schedule: while chunk 0's tail groups wait on
            # the input DMAs, run chunk 1/2's early groups whose PSUM banks
            # chunk 0 has already retired (bank b of chunk t may start only
            # after chunk t-1's bank b drained: b0 retires at g4, b1 at g8,
            # b2 at g13, b3/b4 are double-buffered).
            SCHED = [(0, 0, 12), (1, 0, 7), (2, 0, 3), (1, 7, 12),
                     (0, 12, NG), (2, 3, 5), (1, 12, NG), (2, 5, NG),
                     (3, 0, NG)]
            assert sorted((t, j) for t, j0, j1 in SCHED
                          for j in range(j0, j1)) == \
                sorted((t, j) for t in range(TCH) for j in range(NG))
            # static safety: chunk t's first touch of bank b must come after
            # the bank's previous user fully retired (PE FIFO deadlocks
            # otherwise, since the drain it waits on would be emitted later)
            emit_order = [(t, s // 512) for (t, j0, j1) in SCHED
                          for j in range(j0, j1)
                          for (i, s, e) in segs_by_group[j]]
            first_touch, retire_pos, cnt = {}, {}, {}
            for pos, (t, b) in enumerate(emit_order):
                first_touch.setdefault((t, b), pos)
                cnt[(t, b)] = cnt.get((t, b), 0) + 1
                if cnt[(t, b)] == bank_total[b]:
                    retire_pos[(t, b)] = pos
            nbufs = {0: 1, 1: 1, 2: 2, 3: 2, 4: 2}
            for (t, b), pos in first_touch.items():
                if t >= nbufs[b]:
                    assert retire_pos[(t - nbufs[b], b)] < pos, (t, b)

            pts, ots = {}, {}
            bank_done = {}
            drained = {t: set() for t in range(TCH)}
            drain_eng = {0: 'v', 1: 's', 2: 'v', 3: 's', 4: 'v'}

            # stores spread across the three rings so no single ring's
            # backlog delays an ot-buffer reuse or drain
            store_ring0 = {0: nc.sync, 1: nc.gpsimd, 2: nc.sync, 3: nc.gpsimd}
            store_ring1 = {0: nc.scalar, 1: nc.scalar, 2: nc.gpsimd,
                           3: nc.scalar}

            nsched = 0
            for (t, j0, j1) in SCHED:
                for j in range(j0, j1):
                    # two absorbed warm matmuls ahead of each load-gated
                    # group keep PE duty high enough for the HAM clock gate
                    # (stop before block (1,7,12): t1's bank-2 shares the
                    # warm tile's memory, a start=True filler would clear it)
                    if nsched < 22:
                        nsched += 1
                        for _ in range(2):
                            nc.tensor.matmul(warm[:, 0:P], ident[:], ident[:],
                                             start=True, stop=True)
                    lo2 = 2 * wins[j][0]
                    o = offs[j]
                    for (i, s, e) in segs_by_group[j]:
                        b = s // 512
                        if (t, b) not in pts:
                            pts[(t, b)] = bank_tile(t, b)
                        nc.tensor.matmul(
                            pts[(t, b)][:, s - b * 512:e - b * 512],
                            lhsT(j, t),
                            abt[:, o + s - lo2:o + e - lo2],
                            start=(bank_first[b] == i),
                            stop=(bank_last[b] == i),
                        )
                        bank_done[(t, b)] = bank_done.get((t, b), 0) + 1
                        if bank_done[(t, b)] < bank_total[b]:
                            continue
                        # bank (t, b) retired: drain it now, store halves as
                        # soon as their banks are all in SBUF
                        if t not in ots:
                            ots[t] = opool.tile([P, W_OUT], f16, tag="out",
                                                name=f"out_{t}")
                        dst = ots[t][:, b * 512:b * 512 + min(
                            512, W_OUT - b * 512)]
                        if drain_eng[b] == 'v':
                            nc.vector.tensor_copy(dst, pts[(t, b)][:])
                        else:
                            nc.scalar.copy(dst, pts[(t, b)][:])
                        drained[t].add(b)
                        if b in (0, 1) and {0, 1} <= drained[t]:
                            store_ring0[t].dma_start(y[t, :, 0:1024],
                                                     ots[t][:, 0:1024])
                        if {2, 3, 4} <= drained[t]:
                            store_ring1[t].dma_start(y[t, :, 1024:W_OUT],
                                                     ots[t][:, 1024:W_OUT])

    nc.compile()
    return nc


def kernel(**inputs):
    import ml_dtypes

    x = np.ascontiguousarray(np.asarray(inputs["x"], np.float32))
    B, C, T, F = x.shape
    assert (B, C, F) == (4, _C, _F), (B, C, F)
    TS = T // _N_CORES

    A, const = _fold_matrix(
        inputs["pre_w"], inputs["pre_b"], inputs["post_w"], inputs["post_b"],
        inputs["idx"], inputs["melw"], inputs["mask"], inputs["ola_window"],
    )
    wins, offs, tw, ab, segs, bank_first, bank_last = _plan(A, const)

    key = (tw, tuple(wins))
    if key not in _PROGRAM_CACHE:
        _PROGRAM_CACHE[key] = _build_program(tw, wins, offs, segs,
                                             bank_first, bank_last)
    nc = _PROGRAM_CACHE[key]

    # host pre-shard: contraction-major x layout per core.
    # xq[ci, f, b, t] with f padded to 1088 (bias row at f=1025 == 1.0)
    NGG = _NG * _G
    xq = np.zeros((_C, NGG, B, T), np.float32)
    xq[:, :F] = x.transpose(1, 3, 0, 2)
    xq[0, F] = 1.0
    # [NG, G, C, B, T] -> partitions p = 2*f_off + ci
    xq = xq.reshape(_C, _NG, _G, B, T).transpose(1, 2, 0, 3, 4)
    x8 = (xq[:_FP8_GROUPS] * _FP8_SCALE).astype(ml_dtypes.float8_e3m4)
    x16 = xq[_FP8_GROUPS:].astype(ml_dtypes.bfloat16)

    in_maps = []
    for m in range(_N_CORES):
        # chunk-major layout: xs[t, p, j*128 + tok_local], chunk t == batch b
        sl8 = x8[:, :, :, :, m * TS:(m + 1) * TS]      # (N8, G, C, B, TS)
        sl16 = x16[:, :, :, :, m * TS:(m + 1) * TS]
        in_maps.append({
            "xs8": np.ascontiguousarray(
                sl8.transpose(3, 1, 2, 0, 4)           # (B, G, C, N8, TS)
                   .reshape(_TCH, _P, _FP8_GROUPS * _P)),
            "xs16": np.ascontiguousarray(
                sl16.transpose(3, 1, 2, 0, 4)
                    .reshape(_TCH, _P, (_NG - _FP8_GROUPS) * _P)),
            "ab": ab,
        })

    try:
        import antenv.axon_hooks  # noqa: F401
    except ImportError:
        import sys
        import types
        import antenv
        stub = types.ModuleType("antenv.axon_hooks")
        stub.get_axon_ntff_profile_hook = lambda: None
        stub.set_axon_ntff_profile_hook = lambda h: None
        sys.modules["antenv.axon_hooks"] = stub
        antenv.axon_hooks = stub

    from concourse.bass_utils import run_bass_kernel_spmd
    res = run_bass_kernel_spmd(nc, in_maps, core_ids=list(range(_N_CORES)))
    globals()["_LAST_RESULT"] = res

    out = np.empty((B, _C, T, F), np.float32)
    for m in range(_N_CORES):
        ym = res.results[m]["y"].astype(np.float32)     # (TCH, P, 2050)
        ym = ym.reshape(B, TS, F, _C)                   # tok=(b, t_local)
        out[:, :, m * TS:(m + 1) * TS, :] = ym.transpose(0, 3, 1, 2)
    return out
